# revision 18
# baseline (speedup 1.0000x reference)
"""Trainium2 Bass kernel for nn_NeuralODEModel (dense MLP Neural ODE).

Reference computation (fp32):
    h0 = x[:, 0, :] @ Wi + bi                      # [B, H]
    f(h) = gelu(gelu(gelu(h@W1+b1)@W2+b2)@W3+b3)   # exact (erf) gelu
    15 RK4 (3/8-rule) steps with dt = 1/15
    out = gelu(h@Wo1+bo1) @ Wo2 + bo2              # [B, 64]

Strategy: pure data parallel over 8 NeuronCores (batch 2048 -> 256/core).
All weights + state live in SBUF for the whole integration. Activations
feature-major ([128 part, chunk, batch]); every linear layer is
out_T[m] = sum_g W[:,2g:2g+2,mblk].T @ act[:,2g:2g+2,:] using fp8-e4m3
matmuls in DoubleRow perf mode (2 features per PE cell -> 256-feature
contraction per matmul, ~2x f32r throughput). Weights are pre-scaled by
2^12 host-side so their values sit in e4m3's normal range; the scalar
engine un-scales (scale=2^-12) while applying bias + exact-erf gelu
straight out of PSUM. Accumulation is fp32 in PSUM throughout.

Precision: the carried state h stays fp32; k_i (f-eval outputs) stay
fp32 for the RK linear combinations (vector engine); only matmul inputs
(h, u2, u3, u4 and the two hidden-layer activations of each f-eval) are
rounded to fp8. The init layer (x@Wi) and the output head run in f32r.
Numpy simulation of this exact scheme: rel err ~1.4e-3 (gate 2e-2).
"""

import sys

for _p in ("/opt/trn_rl_repo",):
    if _p not in sys.path:
        sys.path.insert(0, _p)

import numpy as np
import ml_dtypes

import concourse.bacc as bacc
import concourse.tile as tile
import concourse.mybir as mybir
import concourse.hw_specs as hw_specs
from concourse.bass_utils import run_bass_kernel_spmd

# Calibrate the Tile scheduler's cost model to measured DoubleRow behavior:
# fp8 DoubleRow matmuls at FD=256 run ~109 ns back-to-back on HW (the model's
# 0.5 cyc/row @2.4GHz = 53 ns is 2x optimistic), and chained accumulations
# into the same PSUM bank run at full rate (the 173 ns access-latency penalty
# makes the scheduler spread each accumulation group over ~2.5 us, which
# delays the gelu -> u-combo chain every f-eval boundary). Scheduling only;
# numerics are unaffected.
hw_specs.TRN2Spec.PE_CYCLE = 1e9 / 1.2e9
hw_specs.TRN2Spec.PE_SBUF_ACCESS_LATENCY_NS = 30.0

B, S, D_IN, H, D_OUT = 2048, 16, 512, 1024, 64
HID2 = H // 2                 # 512 (head hidden)
N_CORES = 8
BL = B // N_CORES             # 256 per-core batch (matmul moving free dim)
NSTEPS = S - 1                # 15
DT = 1.0 / NSTEPS
P = 128
KH = H // P                   # 8 feature chunks
KG = KH // 2                  # 4 double-row groups
KI = D_IN // P                # 4
KO = HID2 // P                # 4
WSCALE = 2.0 ** 12            # fp8 weight pre-scale (max |W|*4096 = 128 < 240)
SINV = 1.0 / WSCALE

F32 = mybir.dt.float32
F32R = mybir.dt.float32r
FP8 = mybir.dt.float8e4
GELU = mybir.ActivationFunctionType.Gelu
IDENT = mybir.ActivationFunctionType.Identity
DR = mybir.MatmulPerfMode.DoubleRow
MULT = mybir.AluOpType.mult
ADD = mybir.AluOpType.add

_CACHE = {}


def _build():
    nc = bacc.Bacc("TRN2", target_bir_lowering=False, debug=False,
                   enable_asserts=False)

    def din(name, shape, dt=F32):
        return nc.dram_tensor(name, shape, dt, kind="ExternalInput")

    xT_d = din("xT", [P, KI, BL])
    Wi_d = din("Wi", [P, KI, H])
    W1_d = din("W1", [P, KH, H], FP8)
    W2_d = din("W2", [P, KH, H], FP8)
    W3_d = din("W3", [P, KH, H], FP8)
    Wo1_d = din("Wo1", [P, KH, HID2])
    Wo2_d = din("Wo2", [P, KO, D_OUT])
    bi_d = din("bi", [P, KH])
    b1_d = din("b1", [P, KH])
    b2_d = din("b2", [P, KH])
    b3_d = din("b3", [P, KH])
    bo1_d = din("bo1", [P, KO])
    bo2_d = din("bo2", [D_OUT, 1])
    out_d = nc.dram_tensor("outT", [D_OUT, BL], F32, kind="ExternalOutput")

    with tile.TileContext(nc) as tc:
        with (
            tc.tile_pool(name="wpool", bufs=1) as wp,
            tc.tile_pool(name="apool", bufs=1) as ap,
            tc.tile_pool(name="pspool", bufs=8, space="PSUM") as pp,
        ):
            Wi = wp.tile([P, KI, H], F32R, tag="Wi")
            W1 = wp.tile([P, KH, H], FP8, tag="W1")
            W2 = wp.tile([P, KH, H], FP8, tag="W2")
            W3 = wp.tile([P, KH, H], FP8, tag="W3")
            Wo1 = wp.tile([P, KH, HID2], F32R, tag="Wo1")
            Wo2 = wp.tile([P, KO, D_OUT], F32R, tag="Wo2")
            bi = wp.tile([P, KH], F32, tag="bi")
            b1 = wp.tile([P, KH], F32, tag="b1")
            b2 = wp.tile([P, KH], F32, tag="b2")
            b3 = wp.tile([P, KH], F32, tag="b3")
            bo1 = wp.tile([P, KO], F32, tag="bo1")
            bo2 = wp.tile([D_OUT, 1], F32, tag="bo2")
            xT = wp.tile([P, KI, BL], F32R, tag="xT")

            # fp32 state + RK combo buffers
            hA = ap.tile([P, KH, BL], F32, tag="hA")    # carried state
            k1 = ap.tile([P, KH, BL], F32, tag="k1")
            k2 = ap.tile([P, KH, BL], F32, tag="k2")
            k3 = ap.tile([P, KH, BL], F32, tag="k3")
            E = ap.tile([P, KH, BL], F32, tag="E")      # k4
            G = ap.tile([P, KH, BL], F32, tag="G")      # combo precompute
            hRr = ap.tile([P, KH, BL], F32R, tag="hRr")  # final h for head
            # fp8 matmul-input buffers
            hR8 = ap.tile([P, KH, BL], FP8, tag="hR8")  # rounded h
            X8 = ap.tile([P, KH, BL], FP8, tag="X8")    # u2/u3/u4
            Y8 = ap.tile([P, KH, BL], FP8, tag="Y8")    # layer-1 out
            Z8 = ap.tile([P, KH, BL], FP8, tag="Z8")    # layer-2 out

            # DMAs: xT + Wi + W1 gate the start; slice them so compute can
            # begin after the first slices. Rest in coarse slices.
            nc.sync.dma_start(xT[:], xT_d[:].bitcast(F32R))
            nc.sync.dma_start(bi[:], bi_d[:])
            nc.sync.dma_start(b1[:], b1_d[:])
            # Slices arrive in DESCENDING column order to match the
            # descending consumption order of the compute loops.
            wiw = H // KI
            for jj in range(KH):
                j = KH - 1 - jj
                if j >= KH - KI:
                    ji = j - (KH - KI)
                    nc.sync.dma_start(
                        Wi[:, :, ji * wiw:(ji + 1) * wiw],
                        Wi_d[:, :, ji * wiw:(ji + 1) * wiw].bitcast(F32R))
                nc.sync.dma_start(W1[:, :, j * P:(j + 1) * P],
                                  W1_d[:, :, j * P:(j + 1) * P])
            nc.sync.dma_start(b2[:], b2_d[:])
            for jj in range(KH):
                j = KH - 1 - jj
                nc.sync.dma_start(W2[:, j], W2_d[:, j])
            nc.sync.dma_start(b3[:], b3_d[:])
            for jj in range(KH):
                j = KH - 1 - jj
                nc.sync.dma_start(W3[:, j], W3_d[:, j])
            nc.sync.dma_start(bo1[:], bo1_d[:])
            nc.sync.dma_start(Wo1[:], Wo1_d[:].bitcast(F32R))
            nc.sync.dma_start(Wo2[:], Wo2_d[:].bitcast(F32R))
            nc.sync.dma_start(bo2[:], bo2_d[:])

            stt = nc.vector.scalar_tensor_tensor

            def layer8(dst, W, bias, src):
                """fp8 DoubleRow layer: dst = gelu(src@W * 2^-12 + b).

                Emission order is DESCENDING everywhere: output chunks are
                produced m7->m0 and input groups consumed g3->g0, in two
                waves of 4 chunks with the group loop outermost. PE runs in
                program order, so this aligns production with consumption:
                the first-produced gelu outputs (m7, m6) feed the
                first-consumed input group (g3) of the next layer, and the
                last-produced pair (m1, m0) is only needed ~1.45us into the
                next layer -- enough runway to cover the gelu tail plus the
                u-combo on the vector engine at f-eval boundaries.
                """
                for w in range(2):
                    ms = [7 - 4 * w - i for i in range(4)]
                    pss = {m: pp.tile([P, BL], F32, tag="ps", name="ps")
                           for m in ms}
                    for gi in range(KG):
                        g = KG - 1 - gi
                        for m in ms:
                            nc.tensor.matmul(
                                pss[m][:],
                                W[:, 2 * g:2 * g + 2, m * P:(m + 1) * P],
                                src[:, 2 * g:2 * g + 2, :],
                                start=(gi == 0), stop=(gi == KG - 1),
                                perf_mode=DR)
                    for m in ms:
                        nc.scalar.activation(dst[:, m, :], pss[m][:], GELU,
                                             bias=bias[:, m:m + 1], scale=SINV)

            # ---- init: h0 = x @ Wi + bi (f32r), plus fp8 copy ----
            for m in range(KH - 1, -1, -1):
                ps = pp.tile([P, BL], F32, tag="ps")
                for k in range(KI):
                    nc.tensor.matmul(
                        ps[:], Wi[:, k, m * P:(m + 1) * P], xT[:, k, :],
                        start=(k == 0), stop=(k == KI - 1))
                nc.scalar.activation(hA[:, m, :], ps[:], IDENT,
                                     bias=bi[:, m:m + 1], scale=1.0)
                nc.vector.tensor_copy(hR8[:, m, :], hA[:, m, :])

            for step in range(NSTEPS):
                last = step == NSTEPS - 1
                # ---- k1 = f(h) ----
                layer8(Y8, W1, b1, hR8)
                layer8(Z8, W2, b2, Y8)
                layer8(k1, W3, b3, Z8)
                # u2 = h + dt/3*k1 -> X8 (per double-row group, pipelined)
                for g in range(KG - 1, -1, -1):
                    s = slice(2 * g, 2 * g + 2)
                    stt(X8[:, s, :], k1[:, s, :], DT / 3.0, hA[:, s, :],
                        MULT, ADD)
                # ---- k2 = f(u2) ----
                layer8(Y8, W1, b1, X8)
                # G = h - dt/3*k1 (hidden under k2's matmuls; gpsimd keeps
                # the DVE FIFO free for the critical u-combos)
                stt(G[:], k1[:], -DT / 3.0, hA[:], MULT, ADD)
                layer8(Z8, W2, b2, Y8)
                layer8(k2, W3, b3, Z8)
                # u3 = G + dt*k2 -> X8
                for g in range(KG - 1, -1, -1):
                    s = slice(2 * g, 2 * g + 2)
                    stt(X8[:, s, :], k2[:, s, :], DT, G[:, s, :], MULT, ADD)
                # ---- k3 = f(u3) ----
                layer8(Y8, W1, b1, X8)
                # G = h + dt*(k1-k2) (hidden); acc1 moved up here so the
                # DVE FIFO is clear when the u4 inputs arrive
                stt(G[:], k2[:], -1.0, k1[:], MULT, ADD)
                stt(G[:], G[:], DT, hA[:], MULT, ADD)
                stt(k1[:], k2[:], 3.0, k1[:], MULT, ADD)
                layer8(Z8, W2, b2, Y8)
                layer8(k3, W3, b3, Z8)
                # u4 = G + dt*k3 -> X8
                for g in range(KG - 1, -1, -1):
                    s = slice(2 * g, 2 * g + 2)
                    stt(X8[:, s, :], k3[:, s, :], DT, G[:, s, :], MULT, ADD)
                # ---- k4 = f(u4) ----
                layer8(Y8, W1, b1, X8)
                # acc += 3k3 -> k1; hA += dt/8*acc (hidden under k4)
                stt(k1[:], k3[:], 3.0, k1[:], MULT, ADD)
                stt(hA[:], k1[:], DT / 8.0, hA[:], MULT, ADD)
                layer8(Z8, W2, b2, Y8)
                layer8(E, W3, b3, Z8)                   # k4 (fp32)
                # h' = hA + dt/8*k4: fp8 copy for next step's k1 (critical),
                # then the fp32 state update. Last step: f32r copy for head.
                if not last:
                    for g in range(KG - 1, -1, -1):
                        s = slice(2 * g, 2 * g + 2)
                        stt(hR8[:, s, :], E[:, s, :], DT / 8.0, hA[:, s, :],
                            MULT, ADD)
                    stt(hA[:], E[:], DT / 8.0, hA[:], MULT, ADD)
                else:
                    for g in range(KG - 1, -1, -1):
                        s = slice(2 * g, 2 * g + 2)
                        stt(hRr[:, s, :], E[:, s, :], DT / 8.0, hA[:, s, :],
                            MULT, ADD)

            # ---- head: out = gelu(h@Wo1+bo1) @ Wo2 + bo2 (f32r) ----
            o1 = ap.tile([P, KO, BL], F32R, tag="o1")
            for m in range(KO):
                ps = pp.tile([P, BL], F32, tag="ps")
                for ki in range(KH):
                    k = KH - 1 - ki    # hRr chunks were produced descending
                    nc.tensor.matmul(
                        ps[:], Wo1[:, k, m * P:(m + 1) * P], hRr[:, k, :],
                        start=(ki == 0), stop=(ki == KH - 1))
                nc.scalar.activation(o1[:, m, :], ps[:], GELU,
                                     bias=bo1[:, m:m + 1], scale=1.0)
            outT = ap.tile([D_OUT, BL], F32, tag="outT")
            ps = pp.tile([P, BL], F32, tag="ps")
            for k in range(KO):
                nc.tensor.matmul(ps[:D_OUT, :], Wo2[:, k, :], o1[:, k, :],
                                 start=(k == 0), stop=(k == KO - 1))
            nc.vector.tensor_add(outT[:], ps[:D_OUT, :],
                                 bo2[:, 0:1].to_broadcast((D_OUT, BL)))
            nc.sync.dma_start(out_d[:], outT[:])

    nc.compile()
    return nc


def _shard_inputs(inputs):
    """Host-side reshape into the SBUF layouts; returns per-core in_maps."""
    f = np.float32
    E4 = ml_dtypes.float8_e4m3

    def fm(w, kin, n, dt=f, scale=1.0):  # [kin*P, n] -> [P, kin, n]
        a = np.asarray(w, dtype=f) * scale
        a = a.reshape(kin, P, n).transpose(1, 0, 2)
        return np.ascontiguousarray(a).astype(dt)

    def bv(b, kout):             # [kout*P] -> [P, kout]
        return np.ascontiguousarray(np.asarray(b, dtype=f).reshape(kout, P).T)

    shared = {
        "Wi": fm(inputs["Wi"], KI, H),
        "W1": fm(inputs["W1"], KH, H, E4, WSCALE),
        "W2": fm(inputs["W2"], KH, H, E4, WSCALE),
        "W3": fm(inputs["W3"], KH, H, E4, WSCALE),
        "Wo1": fm(inputs["Wo1"], KH, HID2),
        "Wo2": fm(inputs["Wo2"], KO, D_OUT),
        "bi": bv(inputs["bi"], KH),
        "b1": bv(inputs["b1"], KH),
        "b2": bv(inputs["b2"], KH),
        "b3": bv(inputs["b3"], KH),
        "bo1": bv(inputs["bo1"], KO),
        "bo2": np.ascontiguousarray(
            np.asarray(inputs["bo2"], dtype=f).reshape(D_OUT, 1)),
    }
    x = np.asarray(inputs["x"], dtype=f)
    in_maps = []
    for c in range(N_CORES):
        x0c = x[c * BL:(c + 1) * BL, 0, :]            # [BL, D_IN]
        xT = np.ascontiguousarray(
            x0c.T.reshape(KI, P, BL).transpose(1, 0, 2))
        in_maps.append({"xT": xT, **shared})
    return in_maps


def run(inputs, trace=False):
    if "nc" not in _CACHE:
        _CACHE["nc"] = _build()
    nc = _CACHE["nc"]
    in_maps = _shard_inputs(inputs)
    res = run_bass_kernel_spmd(nc, in_maps, list(range(N_CORES)), trace=trace)
    out = np.empty((B, D_OUT), dtype=np.float32)
    for c in range(N_CORES):
        out[c * BL:(c + 1) * BL, :] = res.results[c]["outT"].T
    return out, res


def kernel(**inputs):
    out, _ = run(inputs)
    return out


# revision 20
# speedup vs baseline: 1.1798x; 1.1798x over previous
"""Trainium2 Bass kernel for nn_NeuralODEModel (dense MLP Neural ODE).

Reference computation (fp32):
    h0 = x[:, 0, :] @ Wi + bi                      # [B, H]
    f(h) = gelu(gelu(gelu(h@W1+b1)@W2+b2)@W3+b3)   # exact (erf) gelu
    15 RK4 (3/8-rule) steps with dt = 1/15
    out = gelu(h@Wo1+bo1) @ Wo2 + bo2              # [B, 64]

Strategy: pure data parallel over 8 NeuronCores (batch 2048 -> 256/core).
All weights + state live in SBUF for the whole integration. Activations
feature-major ([128 part, chunk, batch]); every linear layer is
out_T[m] = sum_g W[:,2g:2g+2,mblk].T @ act[:,2g:2g+2,:] using fp8-e4m3
matmuls in DoubleRow perf mode (2 features per PE cell -> 256-feature
contraction per matmul, ~2x f32r throughput). Weights are pre-scaled by
2^12 host-side so their values sit in e4m3's normal range; the scalar
engine un-scales (scale=2^-12) while applying bias + exact-erf gelu
straight out of PSUM. Accumulation is fp32 in PSUM throughout.

Precision: the carried state h stays fp32; k_i (f-eval outputs) stay
fp32 for the RK linear combinations (vector engine); only matmul inputs
(h, u2, u3, u4 and the two hidden-layer activations of each f-eval) are
rounded to fp8. The init layer (x@Wi) and the output head run in f32r.
Numpy simulation of this exact scheme: rel err ~1.4e-3 (gate 2e-2).
"""

import sys

for _p in ("/opt/trn_rl_repo",):
    if _p not in sys.path:
        sys.path.insert(0, _p)

import numpy as np
import ml_dtypes

import concourse.bacc as bacc
import concourse.tile as tile
import concourse.mybir as mybir
import concourse.hw_specs as hw_specs
from concourse.bass_utils import run_bass_kernel_spmd

# Calibrate the Tile scheduler's cost model to measured DoubleRow behavior:
# fp8 DoubleRow matmuls at FD=256 run ~109 ns back-to-back on HW (the model's
# 0.5 cyc/row @2.4GHz = 53 ns is 2x optimistic), and chained accumulations
# into the same PSUM bank run at full rate (the 173 ns access-latency penalty
# makes the scheduler spread each accumulation group over ~2.5 us, which
# delays the gelu -> u-combo chain every f-eval boundary). Scheduling only;
# numerics are unaffected.
hw_specs.TRN2Spec.PE_CYCLE = 1e9 / 1.2e9
hw_specs.TRN2Spec.PE_SBUF_ACCESS_LATENCY_NS = 30.0

B, S, D_IN, H, D_OUT = 2048, 16, 512, 1024, 64
HID2 = H // 2                 # 512 (head hidden)
N_CORES = 8
BL = B // N_CORES             # 256 per-core batch (matmul moving free dim)
NSTEPS = S - 1                # 15
DT = 1.0 / NSTEPS
P = 128
KH = H // P                   # 8 feature chunks
KG = KH // 2                  # 4 double-row groups
KI = D_IN // P                # 4
KO = HID2 // P                # 4
WSCALE = 2.0 ** 12            # fp8 weight pre-scale (max |W|*4096 = 128 < 240)
SINV = 1.0 / WSCALE

F32 = mybir.dt.float32
F32R = mybir.dt.float32r
FP8 = mybir.dt.float8e4
GELU = mybir.ActivationFunctionType.Gelu
IDENT = mybir.ActivationFunctionType.Identity
DR = mybir.MatmulPerfMode.DoubleRow
MULT = mybir.AluOpType.mult
ADD = mybir.AluOpType.add

_CACHE = {}


def _build():
    nc = bacc.Bacc("TRN2", target_bir_lowering=False, debug=False,
                   enable_asserts=False)

    def din(name, shape, dt=F32):
        return nc.dram_tensor(name, shape, dt, kind="ExternalInput")

    xT_d = din("xT", [P, KI, BL])
    Wi_d = din("Wi", [P, KI, H])
    W1_d = din("W1", [P, KH, H], FP8)
    W2_d = din("W2", [P, KH, H], FP8)
    W3_d = din("W3", [P, KH, H], FP8)
    Wo1_d = din("Wo1", [P, KH, HID2])
    Wo2_d = din("Wo2", [P, KO, D_OUT])
    bi_d = din("bi", [P, KH])
    b1_d = din("b1", [P, KH])
    b2_d = din("b2", [P, KH])
    b3_d = din("b3", [P, KH])
    bo1_d = din("bo1", [P, KO])
    bo2_d = din("bo2", [D_OUT, 1])
    out_d = nc.dram_tensor("outT", [D_OUT, BL], F32, kind="ExternalOutput")

    with tile.TileContext(nc) as tc:
        with (
            tc.tile_pool(name="wpool", bufs=1) as wp,
            tc.tile_pool(name="apool", bufs=1) as ap,
            tc.tile_pool(name="pspool", bufs=8, space="PSUM") as pp,
        ):
            Wi = wp.tile([P, KI, H], F32R, tag="Wi")
            W1 = wp.tile([P, KH, H], FP8, tag="W1")
            W2 = wp.tile([P, KH, H], FP8, tag="W2")
            W3 = wp.tile([P, KH, H], FP8, tag="W3")
            Wo1 = wp.tile([P, KH, HID2], F32R, tag="Wo1")
            Wo2 = wp.tile([P, KO, D_OUT], F32R, tag="Wo2")
            bi = wp.tile([P, KH], F32, tag="bi")
            b1 = wp.tile([P, KH], F32, tag="b1")
            b2 = wp.tile([P, KH], F32, tag="b2")
            b3 = wp.tile([P, KH], F32, tag="b3")
            bo1 = wp.tile([P, KO], F32, tag="bo1")
            bo2 = wp.tile([D_OUT, 1], F32, tag="bo2")
            xT = wp.tile([P, KI, BL], F32R, tag="xT")

            # fp32 state + RK combo buffers
            hA = ap.tile([P, KH, BL], F32, tag="hA")    # carried state
            k1 = ap.tile([P, KH, BL], F32, tag="k1")
            k2 = ap.tile([P, KH, BL], F32, tag="k2")
            k3 = ap.tile([P, KH, BL], F32, tag="k3")
            E = ap.tile([P, KH, BL], F32, tag="E")      # k4
            G = ap.tile([P, KH, BL], F32, tag="G")      # combo precompute
            hRr = ap.tile([P, KH, BL], F32R, tag="hRr")  # final h for head
            # fp8 matmul-input buffers
            hR8 = ap.tile([P, KH, BL], FP8, tag="hR8")  # rounded h
            X8 = ap.tile([P, KH, BL], FP8, tag="X8")    # u2/u3/u4
            Y8 = ap.tile([P, KH, BL], FP8, tag="Y8")    # layer-1 out
            Z8 = ap.tile([P, KH, BL], FP8, tag="Z8")    # layer-2 out

            # DMAs: xT + Wi + W1 gate the start; slice them so compute can
            # begin after the first slices. Rest in coarse slices.
            nc.sync.dma_start(xT[:], xT_d[:].bitcast(F32R))
            nc.sync.dma_start(bi[:], bi_d[:])
            nc.sync.dma_start(b1[:], b1_d[:])
            wiw = H // KI
            for j in range(KH):
                if j < KI:
                    nc.sync.dma_start(
                        Wi[:, :, j * wiw:(j + 1) * wiw],
                        Wi_d[:, :, j * wiw:(j + 1) * wiw].bitcast(F32R))
                nc.sync.dma_start(W1[:, :, j * P:(j + 1) * P],
                                  W1_d[:, :, j * P:(j + 1) * P])
            nc.sync.dma_start(b2[:], b2_d[:])
            for j in range(KH):
                nc.sync.dma_start(W2[:, j], W2_d[:, j])
            nc.sync.dma_start(b3[:], b3_d[:])
            for j in range(KH):
                nc.sync.dma_start(W3[:, j], W3_d[:, j])
            nc.sync.dma_start(bo1[:], bo1_d[:])
            nc.sync.dma_start(Wo1[:], Wo1_d[:].bitcast(F32R))
            nc.sync.dma_start(Wo2[:], Wo2_d[:].bitcast(F32R))
            nc.sync.dma_start(bo2[:], bo2_d[:])

            stt = nc.vector.scalar_tensor_tensor

            def layer8(dst, W, bias, src):
                """fp8 DoubleRow layer: dst = gelu(src@W * 2^-12 + b).

                Emission order is DESCENDING everywhere: output chunks are
                produced m7->m0 and input groups consumed g3->g0, in two
                waves of 4 chunks with the group loop outermost. PE runs in
                program order, so this aligns production with consumption:
                the first-produced gelu outputs (m7, m6) feed the
                first-consumed input group (g3) of the next layer, and the
                last-produced pair (m1, m0) is only needed ~1.45us into the
                next layer -- enough runway to cover the gelu tail plus the
                u-combo on the vector engine at f-eval boundaries.
                """
                for w in range(2):
                    ms = [7 - 4 * w - i for i in range(4)]
                    pss = {m: pp.tile([P, BL], F32, tag="ps", name="ps")
                           for m in ms}
                    for gi in range(KG):
                        g = KG - 1 - gi
                        for m in ms:
                            nc.tensor.matmul(
                                pss[m][:],
                                W[:, 2 * g:2 * g + 2, m * P:(m + 1) * P],
                                src[:, 2 * g:2 * g + 2, :],
                                start=(gi == 0), stop=(gi == KG - 1),
                                perf_mode=DR)
                    for m in ms:
                        nc.scalar.activation(dst[:, m, :], pss[m][:], GELU,
                                             bias=bias[:, m:m + 1], scale=SINV)

            # ---- init: h0 = x @ Wi + bi (f32r), plus fp8 copy ----
            for m in range(KH - 1, -1, -1):
                ps = pp.tile([P, BL], F32, tag="ps")
                for k in range(KI):
                    nc.tensor.matmul(
                        ps[:], Wi[:, k, m * P:(m + 1) * P], xT[:, k, :],
                        start=(k == 0), stop=(k == KI - 1))
                nc.scalar.activation(hA[:, m, :], ps[:], IDENT,
                                     bias=bi[:, m:m + 1], scale=1.0)
                nc.vector.tensor_copy(hR8[:, m, :], hA[:, m, :])

            for step in range(NSTEPS):
                last = step == NSTEPS - 1
                # ---- k1 = f(h) ----
                layer8(Y8, W1, b1, hR8)
                layer8(Z8, W2, b2, Y8)
                layer8(k1, W3, b3, Z8)
                # u2 = h + dt/3*k1 -> X8 (per double-row group, pipelined)
                for g in range(KG - 1, -1, -1):
                    s = slice(2 * g, 2 * g + 2)
                    stt(X8[:, s, :], k1[:, s, :], DT / 3.0, hA[:, s, :],
                        MULT, ADD)
                # ---- k2 = f(u2) ----
                layer8(Y8, W1, b1, X8)
                # G = h - dt/3*k1 (hidden under k2's matmuls; gpsimd keeps
                # the DVE FIFO free for the critical u-combos)
                stt(G[:], k1[:], -DT / 3.0, hA[:], MULT, ADD)
                layer8(Z8, W2, b2, Y8)
                layer8(k2, W3, b3, Z8)
                # u3 = G + dt*k2 -> X8
                for g in range(KG - 1, -1, -1):
                    s = slice(2 * g, 2 * g + 2)
                    stt(X8[:, s, :], k2[:, s, :], DT, G[:, s, :], MULT, ADD)
                # ---- k3 = f(u3) ----
                layer8(Y8, W1, b1, X8)
                # G = h + dt*(k1-k2) (hidden); acc1 moved up here so the
                # DVE FIFO is clear when the u4 inputs arrive
                stt(G[:], k2[:], -1.0, k1[:], MULT, ADD)
                stt(G[:], G[:], DT, hA[:], MULT, ADD)
                stt(k1[:], k2[:], 3.0, k1[:], MULT, ADD)
                layer8(Z8, W2, b2, Y8)
                layer8(k3, W3, b3, Z8)
                # u4 = G + dt*k3 -> X8
                for g in range(KG - 1, -1, -1):
                    s = slice(2 * g, 2 * g + 2)
                    stt(X8[:, s, :], k3[:, s, :], DT, G[:, s, :], MULT, ADD)
                # ---- k4 = f(u4) ----
                layer8(Y8, W1, b1, X8)
                # acc += 3k3 -> k1; hA += dt/8*acc (hidden under k4)
                stt(k1[:], k3[:], 3.0, k1[:], MULT, ADD)
                stt(hA[:], k1[:], DT / 8.0, hA[:], MULT, ADD)
                layer8(Z8, W2, b2, Y8)
                layer8(E, W3, b3, Z8)                   # k4 (fp32)
                # h' = hA + dt/8*k4: fp8 copy for next step's k1 (critical),
                # then the fp32 state update. Last step: f32r copy for head.
                if not last:
                    for g in range(KG - 1, -1, -1):
                        s = slice(2 * g, 2 * g + 2)
                        stt(hR8[:, s, :], E[:, s, :], DT / 8.0, hA[:, s, :],
                            MULT, ADD)
                    stt(hA[:], E[:], DT / 8.0, hA[:], MULT, ADD)
                else:
                    for g in range(KG - 1, -1, -1):
                        s = slice(2 * g, 2 * g + 2)
                        stt(hRr[:, s, :], E[:, s, :], DT / 8.0, hA[:, s, :],
                            MULT, ADD)

            # ---- head: out = gelu(h@Wo1+bo1) @ Wo2 + bo2 (f32r) ----
            o1 = ap.tile([P, KO, BL], F32R, tag="o1")
            for m in range(KO):
                ps = pp.tile([P, BL], F32, tag="ps")
                for k in range(KH):
                    nc.tensor.matmul(
                        ps[:], Wo1[:, k, m * P:(m + 1) * P], hRr[:, k, :],
                        start=(k == 0), stop=(k == KH - 1))
                nc.scalar.activation(o1[:, m, :], ps[:], GELU,
                                     bias=bo1[:, m:m + 1], scale=1.0)
            outT = ap.tile([D_OUT, BL], F32, tag="outT")
            ps = pp.tile([P, BL], F32, tag="ps")
            for k in range(KO):
                nc.tensor.matmul(ps[:D_OUT, :], Wo2[:, k, :], o1[:, k, :],
                                 start=(k == 0), stop=(k == KO - 1))
            nc.vector.tensor_add(outT[:], ps[:D_OUT, :],
                                 bo2[:, 0:1].to_broadcast((D_OUT, BL)))
            nc.sync.dma_start(out_d[:], outT[:])

    nc.compile()
    return nc


def _shard_inputs(inputs):
    """Host-side reshape into the SBUF layouts; returns per-core in_maps."""
    f = np.float32
    E4 = ml_dtypes.float8_e4m3

    def fm(w, kin, n, dt=f, scale=1.0):  # [kin*P, n] -> [P, kin, n]
        a = np.asarray(w, dtype=f) * scale
        a = a.reshape(kin, P, n).transpose(1, 0, 2)
        return np.ascontiguousarray(a).astype(dt)

    def bv(b, kout):             # [kout*P] -> [P, kout]
        return np.ascontiguousarray(np.asarray(b, dtype=f).reshape(kout, P).T)

    shared = {
        "Wi": fm(inputs["Wi"], KI, H),
        "W1": fm(inputs["W1"], KH, H, E4, WSCALE),
        "W2": fm(inputs["W2"], KH, H, E4, WSCALE),
        "W3": fm(inputs["W3"], KH, H, E4, WSCALE),
        "Wo1": fm(inputs["Wo1"], KH, HID2),
        "Wo2": fm(inputs["Wo2"], KO, D_OUT),
        "bi": bv(inputs["bi"], KH),
        "b1": bv(inputs["b1"], KH),
        "b2": bv(inputs["b2"], KH),
        "b3": bv(inputs["b3"], KH),
        "bo1": bv(inputs["bo1"], KO),
        "bo2": np.ascontiguousarray(
            np.asarray(inputs["bo2"], dtype=f).reshape(D_OUT, 1)),
    }
    x = np.asarray(inputs["x"], dtype=f)
    in_maps = []
    for c in range(N_CORES):
        x0c = x[c * BL:(c + 1) * BL, 0, :]            # [BL, D_IN]
        xT = np.ascontiguousarray(
            x0c.T.reshape(KI, P, BL).transpose(1, 0, 2))
        in_maps.append({"xT": xT, **shared})
    return in_maps


def run(inputs, trace=False):
    if "nc" not in _CACHE:
        _CACHE["nc"] = _build()
    nc = _CACHE["nc"]
    in_maps = _shard_inputs(inputs)
    res = run_bass_kernel_spmd(nc, in_maps, list(range(N_CORES)), trace=trace)
    out = np.empty((B, D_OUT), dtype=np.float32)
    for c in range(N_CORES):
        out[c * BL:(c + 1) * BL, :] = res.results[c]["outT"].T
    return out, res


def kernel(**inputs):
    out, _ = run(inputs)
    return out
